# revision 15
# baseline (speedup 1.0000x reference)
"""GRU cell kernel for Trainium2, data-parallel over 8 NeuronCores.

Math (per reference):
    z = sigmoid(x @ wz.T + h @ uz.T + bz)
    r = sigmoid(x @ wr.T + h @ ur.T + br)
    g = tanh(x @ wh.T + (r*h) @ uh.T + bh)
    out = (1-z)*h + z*g = h + z*(g - h)

Everything on-device is computed in TRANSPOSED layout ([feature, row]) so
both matmul operands arrive with the contraction dim on partitions.

Mixed precision: fp8(e4m3) DoubleRow matmuls (2 MACs/cell/cycle, K=256 per
pass) for most k-quarters; the rest run as fp16 (native 1 cyc/row on the PE,
numerically exact at our value range — measured rel err 1.6e-7 on HW, 8x
finer mantissa than bf16 for free). Which k-quarters of each weight matrix
are fp8 was chosen by host simulation (sim matches HW rel err to ~1e-4):
    wr, ur, uh: all 4 quarters fp8 (r-gate error is attenuated by the
        sigmoid slope and diluted through the uh matmul)
    wz, uz, wh: quarters 0-1 fp8, quarters 2-3 fp16 (z-gate errors are
        amplified by (g - h), tanh has slope 1)
All weights are pre-scaled by 32 on host (exact in both formats) so fp8 and
fp16 products share one PSUM accumulation; the activation undoes it with
scale=1/32.

The combine (out = h + z*(g-h)) reads the fp16 copy of h — simulation shows
identical max rel err vs an fp32 h copy, and it removes the baseline's 8MB
fp32 h stream entirely.

Startup: inputs stream on all four DMA queues (sync/scalar/vector/gpsimd)
with the first x8 quarter split into row-slices so the first real matmul
can issue as soon as ~128KB lands; dummy matmuls on zeroed tiles warm the
PE p-state ramp (0.65->2.4GHz over ~3us) during the DMA wait.

Sharding: rows 16384 -> 8 cores x 2048 rows, weights replicated.
"""

import numpy as np
import ml_dtypes
from contextlib import ExitStack

import concourse.bass as bass
import concourse.bacc as bacc
import concourse.mybir as mybir
import concourse.tile as tile
from concourse.bass_utils import run_bass_kernel_spmd

H = 1024
N_ROWS = 16384
NCORES = 8
P = 128
KB = H // P            # 8 contraction blocks of 128
MB = H // P            # 8 output-feature blocks
NQ = 4                 # k-quarters (256 each)
NS = 512               # rows per matmul moving slice (one PSUM bank)
WSCALE = 32.0          # weight pre-scale (exact power of 2)

# fp8 k-quarters per weight matrix (first nq of 4 quarters are fp8; rest fp16)
NQ8 = {"wz": 2, "uz": 2, "wr": 4, "ur": 4, "wh": 2, "uh": 4}
X16B = KB - 2 * min(NQ8["wz"], NQ8["wh"])   # fp16 x blocks needed (tail blocks)
X16O = KB - X16B                            # first fp16 x block index

F16 = mybir.dt.float16
F8 = mybir.dt.float8e4
F32 = mybir.dt.float32
AF = mybir.ActivationFunctionType
DR = mybir.MatmulPerfMode.DoubleRow
f16 = np.float16
f8 = ml_dtypes.float8_e4m3

# Set by test harness to capture a trace; harness-facing default off.
TRACE = False
LAST_RESULT = None


def build_nc(R=N_ROWS // NCORES):
    """Build the per-core Bass program. R rows per core, single chunk."""
    SL = R // NS           # moving slices (4 for R=2048)

    nc = bacc.Bacc(trn_type="TRN2", target_bir_lowering=False,
                   debug=False, enable_asserts=False)

    # All block tensors use "partition-major block layout": [128, nblk, cols]
    # with element (p, k, c) = T[k*128 + p, c]. One DMA descriptor can then
    # cover any k-block range (descriptor processing on the queue engines
    # costs ~0.65us each — fine-grained DMA was the startup limiter).
    x8d = nc.dram_tensor("x8", [P, KB * R], F8, kind="ExternalInput").ap()
    h8d = nc.dram_tensor("h8", [P, KB * R], F8, kind="ExternalInput").ap()
    x16d = nc.dram_tensor("x16", [P, X16B * R], F16, kind="ExternalInput").ap()
    h16d = nc.dram_tensor("h16", [P, KB * R], F16, kind="ExternalInput").ap()
    w8d = {}
    w16d = {}
    for nm, nq in NQ8.items():
        w8d[nm] = nc.dram_tensor(nm + "8", [P, 2 * nq * H], F8,
                                 kind="ExternalInput").ap()
        if nq < NQ:
            w16d[nm] = nc.dram_tensor(nm + "f", [P, 2 * (NQ - nq) * H], F16,
                                      kind="ExternalInput").ap()
    bias = nc.dram_tensor("bias", [P, 3 * MB], F32, kind="ExternalInput").ap()
    outT = nc.dram_tensor("outT", [H, R], F16, kind="ExternalOutput").ap()

    with tile.TileContext(nc) as tc, ExitStack() as ctx:
        wpool = ctx.enter_context(tc.tile_pool(name="w", bufs=3))
        dpool = ctx.enter_context(tc.tile_pool(name="d", bufs=1))
        rpool = ctx.enter_context(tc.tile_pool(name="r", bufs=3))
        gpool = ctx.enter_context(tc.tile_pool(name="g", bufs=3))
        dtpool = ctx.enter_context(tc.tile_pool(name="dt", bufs=SL))
        opool = ctx.enter_context(tc.tile_pool(name="o", bufs=4))
        cpool = ctx.enter_context(tc.tile_pool(name="c", bufs=1))
        pspool = ctx.enter_context(tc.tile_pool(name="ps", bufs=8, space="PSUM"))

        bt = cpool.tile([P, 3 * MB], F32, tag="bias")
        nc.sync.dma_start(bt[:], bias[:])
        # bias column layout: [z:0..7 | r:8..15 | h:16..23]
        GZ, GR, GH = 0, 1, 2
        ISC = 1.0 / WSCALE

        # ---- SBUF data tiles ----
        xt8 = dpool.tile([P, KB, R], F8, tag="x8")
        ht8 = dpool.tile([P, KB, R], F8, tag="h8")
        xt16 = dpool.tile([P, X16B, R], F16, tag="x16")
        ht16 = dpool.tile([P, KB, R], F16, tag="h16")
        rht = dpool.tile([P, KB, R], F8, tag="rh")

        w8t = {}
        w16t = {}
        # Critical path (r-pass m=0): the critical 6MB (wr+ur+x8+h8) is
        # split across the two HW DGE queues in consumption order of the
        # first psum group (sync: wr+h8, scalar: x8+ur), with x8's kq0
        # additionally split into s-slices so the first matmul only waits
        # on 128KB. The gpsimd SWDGE queue is NOT used for data: its
        # software descriptor generation is far too slow.
        w8t["wr"] = wpool.tile([P, KB, H], F8, tag="w8", name="wr8", bufs=2)
        w8t["ur"] = wpool.tile([P, KB, H], F8, tag="w8", name="ur8", bufs=2)
        for kq in range(NQ):
            j = slice(2 * kq, 2 * kq + 2)
            nc.sync.dma_start(w8t["wr"][:, j, :],
                              w8d["wr"][:, 2 * kq * H:(2 * kq + 2) * H])
            nc.scalar.dma_start(xt8[:, j, :],
                                x8d[:, 2 * kq * R:(2 * kq + 2) * R])
            nc.sync.dma_start(ht8[:, j, :],
                              h8d[:, 2 * kq * R:(2 * kq + 2) * R])
            nc.scalar.dma_start(w8t["ur"][:, j, :],
                                w8d["ur"][:, 2 * kq * H:(2 * kq + 2) * H])

        # Warm up the ACT table set (sigmoid_and_others covers tanh too) on an
        # instruction with minimal sync waits — walrus can't attach the
        # PSEUDO_LOAD_ACT_FUNC_SET to an activation that already carries two
        # sem waits ("Too many sync wait commands"). Emitted after the
        # critical DMAs so the 2x1.3us table loads don't delay them.
        warm = cpool.tile([P, 8], F32, tag="warm")
        nc.gpsimd.memset(warm[:], 0.0)
        nc.scalar.activation(warm[:], warm[:], AF.Sigmoid)

        # fp16 h stream (rht multiply from m=0 at ~23us, combine later) on
        # the sync ring: the sync ENGINE only issues doorbells, so blocking
        # on ring space there is free. The scalar engine must stay at <=9
        # upfront doorbells or its later ACT instructions get stuck behind
        # ring-full doorbell stalls (that cost 20us in an earlier rev).
        for m in range(MB):
            nc.sync.dma_start(ht16[:, m, :], h16d[:, m * R:(m + 1) * R])

        # hz-pass weights + fp16 x: streamed during the r-pass.
        for nm in ("wh", "wz", "uz"):
            nq = NQ8[nm]
            w8t[nm] = wpool.tile([P, 2 * nq, H], F8, tag="w8q", name=nm + "8")
            nc.sync.dma_start(w8t[nm][:, :, :], w8d[nm][:, :])
            w16t[nm] = wpool.tile([P, 2 * (NQ - nq), H], F16, tag="wfq",
                                  name=nm + "f")
            nc.sync.dma_start(w16t[nm][:, :, :], w16d[nm][:, :])
        nc.scalar.dma_start(xt16[:, :, :], x16d[:, :])
        # uh8 reuses wr's buffer (tag w8, bufs=2): second physical buffer is
        # free immediately; keep it at the sync-queue tail so it can't
        # head-of-line block the critical prefetch above.
        w8t["uh"] = wpool.tile([P, KB, H], F8, tag="w8", name="uh8", bufs=2)
        nc.sync.dma_start(w8t["uh"][:, :, :], w8d["uh"][:, :])

        # ---- PE p-state warmup ----
        # The PE ramps 0.65 -> 1.2 -> 2.4GHz over ~3us of continuous work.
        # Burn the ramp on dummy matmuls over zeroed tiles while the first
        # real DMAs are still in flight. They accumulate into the first real
        # PSUM tile as complete start/stop groups, so the real group's
        # start=True afterwards is clean.
        wuw = cpool.tile([P, P], F16, tag="wuw")
        wum = cpool.tile([P, NS], F16, tag="wum")
        nc.gpsimd.memset(wuw[:], 0.0)
        nc.gpsimd.memset(wum[:], 0.0)

        pss01 = [[pspool.tile([P, NS], F32, tag="ps", name="ps")
                  for _ in range(SL)] for _ in range(2)]
        for _ in range(6):
            nc.tensor.matmul(pss01[0][0][:], wuw[:], wum[:],
                             start=True, stop=True, skip_group_check=True)

        def pe_filler(ps, n):
            """Keep the PE p-state ramp alive while the startup DMA stream
            catches up: accumulate all-zero products into the live psum
            group (numerically a no-op, 216ns each)."""
            for _ in range(n):
                nc.tensor.matmul(ps[:], wuw[:], wum[:], start=False,
                                 stop=False, skip_group_check=True)

        def mm_fp8(psums, wt, mov, m, nq, start, stop):
            """DoubleRow-accumulate wt.T @ mov for feature block m over
            fp8 k-quarters 0..nq-1."""
            msl = slice(m * P, (m + 1) * P)
            for kq in range(nq):
                for s in range(len(psums)):
                    nc.tensor.matmul(
                        psums[s][:],
                        wt[:, 2 * kq:2 * kq + 2, msl],
                        mov[:, 2 * kq:2 * kq + 2, s * NS:(s + 1) * NS],
                        start=start and kq == 0,
                        stop=stop and kq == nq - 1,
                        perf_mode=DR,
                    )

        def mm_f16(psums, wt, mov, m, nk, start, stop, mov_off=0):
            """fp16-accumulate over nk k-blocks of 128. mov_off: first
            k-block of this weight's fp16 span within the mov tile."""
            msl = slice(m * P, (m + 1) * P)
            for k in range(nk):
                for s in range(len(psums)):
                    nc.tensor.matmul(
                        psums[s][:],
                        wt[:, k, msl],
                        mov[:, mov_off + k, s * NS:(s + 1) * NS],
                        start=start and k == 0,
                        stop=stop and k == nk - 1,
                    )

        # ---- r pass ----
        # wr/ur interleaved per kq: matches the arrival order of the DMA
        # queues so the m-groups consume data as it lands. The FIRST two
        # m-blocks are fused into one kq-interleaved wave: during the 0-17us
        # window the critical 6MB is still streaming in and a single m-group
        # (6.9us of matmuls) cannot cover the delivery time; two can. Later
        # groups stay single-m so their ACT drain pipelines under the next
        # group's matmuls (4+4 PSUM bank split — fusing ALL pairs regresses).
        def r_mms(ms, pss, fill=False):
            for kq in range(NQ):
                j = slice(2 * kq, 2 * kq + 2)
                for mi, m in enumerate(ms):
                    msl = slice(m * P, (m + 1) * P)
                    for s in range(SL):
                        nc.tensor.matmul(
                            pss[mi][s][:], w8t["wr"][:, j, msl],
                            xt8[:, j, s * NS:(s + 1) * NS],
                            start=kq == 0, stop=False, perf_mode=DR)
                if fill:
                    pe_filler(pss[0][0], 2)
                for mi, m in enumerate(ms):
                    msl = slice(m * P, (m + 1) * P)
                    for s in range(SL):
                        nc.tensor.matmul(
                            pss[mi][s][:], w8t["ur"][:, j, msl],
                            ht8[:, j, s * NS:(s + 1) * NS],
                            start=False, stop=kq == NQ - 1, perf_mode=DR)
                if fill and kq < NQ - 1:
                    pe_filler(pss[0][0], 2)

        def r_acts(ms, pss):
            for mi, m in enumerate(ms):
                for s in range(SL):
                    rt = rpool.tile([P, NS], F16, tag="r")
                    nc.scalar.activation(rt[:], pss[mi][s][:], AF.Sigmoid,
                                         bias=bt[:, GR * MB + m: GR * MB + m + 1],
                                         scale=ISC)
                    nc.vector.tensor_mul(
                        rht[:, m, s * NS:(s + 1) * NS], rt[:],
                        ht16[:, m, s * NS:(s + 1) * NS])

        r_mms([0, 1], pss01, fill=True)
        r_acts([0, 1], pss01)
        for m in range(2, MB):
            ps = [pspool.tile([P, NS], F32, tag="ps", name="ps") for _ in range(SL)]
            r_mms([m], [ps])
            r_acts([m], [ps])

        # ---- fused h~ / z pass + combine ----
        NF16 = {nm: 2 * (NQ - NQ8[nm]) for nm in NQ8}   # fp16 k-blocks
        for m in range(MB):
            msl = slice(m * P, (m + 1) * P)

            psA = [pspool.tile([P, NS], F32, tag="ps", name="psA") for _ in range(SL)]
            mm_fp8(psA, w8t["wh"], xt8, m, NQ8["wh"], start=True, stop=False)
            mm_f16(psA, w16t["wh"], xt16, m, NF16["wh"], start=False,
                   stop=False, mov_off=2 * NQ8["wh"] - X16O)
            mm_fp8(psA, w8t["uh"], rht, m, NQ8["uh"], start=False, stop=True)
            dts = []
            for s in range(SL):
                ssl = slice(s * NS, (s + 1) * NS)
                gt = gpool.tile([P, NS], F16, tag="g")
                nc.scalar.activation(gt[:], psA[s][:], AF.Tanh,
                                     bias=bt[:, GH * MB + m: GH * MB + m + 1],
                                     scale=ISC)
                # g - h does not depend on z: hoist it ahead of the z matmuls
                dt = dtpool.tile([P, NS], F32, tag="dt")
                nc.vector.tensor_sub(dt[:], gt[:], ht16[:, m, ssl])
                dts.append(dt)

            if m < MB - 1:
                psB = [pspool.tile([P, NS], F32, tag="ps", name="psB")
                       for _ in range(SL)]
                mm_fp8(psB, w8t["wz"], xt8, m, NQ8["wz"], start=True, stop=False)
                mm_f16(psB, w16t["wz"], xt16, m, NF16["wz"], start=False,
                       stop=False, mov_off=2 * NQ8["wz"] - X16O)
                mm_fp8(psB, w8t["uz"], ht8, m, NQ8["uz"], start=False, stop=False)
                mm_f16(psB, w16t["uz"], ht16, m, NF16["uz"], start=False,
                       stop=True, mov_off=2 * NQ8["uz"])
                for s in range(SL):
                    ssl = slice(s * NS, (s + 1) * NS)
                    zt = rpool.tile([P, NS], F16, tag="z")
                    nc.scalar.activation(zt[:], psB[s][:], AF.Sigmoid,
                                         bias=bt[:, GZ * MB + m: GZ * MB + m + 1],
                                         scale=ISC)
                    ot = opool.tile([P, NS], F16, tag="o")
                    # z*(g-h) ; h + z*(g-h)
                    nc.vector.tensor_mul(dts[s][:], zt[:], dts[s][:])
                    nc.vector.tensor_add(ot[:], dts[s][:], ht16[:, m, ssl])
                    nc.sync.dma_start(outT[msl, ssl], ot[:])
            else:
                # last m: complete each s-slice fully so the z->combine->dma
                # chains drain during (not after) the matmul stream; the last
                # s-slice is split in half so the final drain chain is short.
                def z_chain(ps, csl, width):
                    for kq in range(NQ8["wz"]):
                        nc.tensor.matmul(ps[:], w8t["wz"][:, 2*kq:2*kq+2, msl],
                                         xt8[:, 2*kq:2*kq+2, csl],
                                         start=kq == 0, stop=False,
                                         perf_mode=DR)
                    for k in range(NF16["wz"]):
                        nc.tensor.matmul(ps[:], w16t["wz"][:, k, msl],
                                         xt16[:, 2*NQ8["wz"]-X16O+k, csl],
                                         start=False, stop=False)
                    for kq in range(NQ8["uz"]):
                        nc.tensor.matmul(ps[:], w8t["uz"][:, 2*kq:2*kq+2, msl],
                                         ht8[:, 2*kq:2*kq+2, csl],
                                         start=False, stop=False, perf_mode=DR)
                    for k in range(NF16["uz"]):
                        nc.tensor.matmul(ps[:], w16t["uz"][:, k, msl],
                                         ht16[:, 2*NQ8["uz"]+k, csl],
                                         start=False, stop=k == NF16["uz"]-1)
                    zt = rpool.tile([P, width], F16, tag="z")
                    nc.scalar.activation(zt[:], ps[:], AF.Sigmoid,
                                         bias=bt[:, GZ * MB + m: GZ * MB + m + 1],
                                         scale=ISC)
                    s = csl.start // NS
                    dsl = slice(csl.start - s * NS, csl.start - s * NS + width)
                    ot = opool.tile([P, width], F16, tag="o")
                    nc.vector.tensor_mul(dts[s][:, dsl], zt[:], dts[s][:, dsl])
                    nc.vector.tensor_add(ot[:], dts[s][:, dsl], ht16[:, m, csl])
                    nc.sync.dma_start(outT[msl, csl], ot[:])

                for s in range(SL - 1):
                    ps = pspool.tile([P, NS], F32, tag="ps", name="psB")
                    z_chain(ps[:], slice(s * NS, (s + 1) * NS), NS)
                # final slice: two sequential half-bank groups so the very
                # last act->combine->dma chain is half as long
                HNS = NS // 2
                ps = pspool.tile([P, NS], F32, tag="ps", name="psB")
                for hf in range(2):
                    c0 = (SL - 1) * NS + hf * HNS
                    z_chain(ps[:, hf * HNS:(hf + 1) * HNS],
                            slice(c0, c0 + HNS), HNS)

    nc.compile()
    return nc


_NC_CACHE = {}


def _get_nc(R):
    if R not in _NC_CACHE:
        _NC_CACHE[R] = build_nc(R)
    return _NC_CACHE[R]


def blockify(a):
    """[nb*128, C] -> partition-major block layout [128, nb*C]."""
    nb = a.shape[0] // P
    return np.ascontiguousarray(
        a.reshape(nb, P, -1).transpose(1, 0, 2).reshape(P, -1))


def make_in_maps(update, hidden, wz, uz, bz, wr, ur, br, wh, uh, bh,
                 ncores=NCORES):
    wmap = {}
    for nm, w in (("wz", wz), ("uz", uz), ("wr", wr), ("ur", ur),
                  ("wh", wh), ("uh", uh)):
        wT = np.ascontiguousarray(np.asarray(w, np.float32).T) * WSCALE
        nq = NQ8[nm]
        wmap[nm + "8"] = blockify(wT[:nq * 2 * P].astype(f8))
        if nq < NQ:
            wmap[nm + "f"] = blockify(wT[nq * 2 * P:].astype(f16))
    bias = np.empty((P, 3 * MB), np.float32)
    for g, b in enumerate((bz, br, bh)):
        bias[:, g * MB:(g + 1) * MB] = np.asarray(b, np.float32).reshape(MB, P).T
    rows = update.shape[0]
    rc = rows // ncores
    in_maps = []
    for i in range(ncores):
        sl = slice(i * rc, (i + 1) * rc)
        xT = np.ascontiguousarray(np.asarray(update[sl], np.float32).T)
        hT = np.ascontiguousarray(np.asarray(hidden[sl], np.float32).T)
        in_maps.append(dict(
            x8=blockify(xT.astype(f8)), x16=blockify(xT[X16O * P:].astype(f16)),
            h8=blockify(hT.astype(f8)), h16=blockify(hT.astype(f16)),
            bias=bias, **wmap))
    return in_maps


def kernel(update, hidden, wz, uz, bz, wr, ur, br, wh, uh, bh):
    global LAST_RESULT
    update = np.asarray(update)
    hidden = np.asarray(hidden)
    R = update.shape[0] // NCORES
    nc = _get_nc(R)
    in_maps = make_in_maps(update, hidden, wz, uz, bz, wr, ur, br, wh, uh, bh)
    res = run_bass_kernel_spmd(nc, in_maps, list(range(NCORES)), trace=TRACE)
    LAST_RESULT = res
    out = np.empty((update.shape[0], H), np.float32)
    for i in range(NCORES):
        out[i * R:(i + 1) * R] = res.results[i]["outT"].T
    return out


# revision 17
# speedup vs baseline: 1.0542x; 1.0542x over previous
"""GRU cell kernel for Trainium2, data-parallel over 8 NeuronCores.

Math (per reference):
    z = sigmoid(x @ wz.T + h @ uz.T + bz)
    r = sigmoid(x @ wr.T + h @ ur.T + br)
    g = tanh(x @ wh.T + (r*h) @ uh.T + bh)
    out = (1-z)*h + z*g = h + z*(g - h)

Everything on-device is computed in TRANSPOSED layout ([feature, row]) so
both matmul operands arrive with the contraction dim on partitions.

Mixed precision: fp8(e4m3) DoubleRow matmuls (2 MACs/cell/cycle, K=256 per
pass) for most k-quarters; the rest run as fp16 (native 1 cyc/row on the PE,
numerically exact at our value range — measured rel err 1.6e-7 on HW, 8x
finer mantissa than bf16 for free). Which k-quarters of each weight matrix
are fp8 was chosen by host simulation (sim matches HW rel err to ~1e-4):
    wr, ur, uh: all 4 quarters fp8 (r-gate error is attenuated by the
        sigmoid slope and diluted through the uh matmul)
    wz, uz, wh: quarters 0-1 fp8, quarters 2-3 fp16 (z-gate errors are
        amplified by (g - h), tanh has slope 1)
All weights are pre-scaled by 32 on host (exact in both formats) so fp8 and
fp16 products share one PSUM accumulation; the activation undoes it with
scale=1/32.

The combine (out = h + z*(g-h)) reads the fp16 copy of h — simulation shows
identical max rel err vs an fp32 h copy, and it removes the baseline's 8MB
fp32 h stream entirely.

Startup: inputs stream on all four DMA queues (sync/scalar/vector/gpsimd)
with the first x8 quarter split into row-slices so the first real matmul
can issue as soon as ~128KB lands; dummy matmuls on zeroed tiles warm the
PE p-state ramp (0.65->2.4GHz over ~3us) during the DMA wait.

Sharding: rows 16384 -> 8 cores x 2048 rows, weights replicated.
"""

import numpy as np
import ml_dtypes
from contextlib import ExitStack

import concourse.bass as bass
import concourse.bacc as bacc
import concourse.mybir as mybir
import concourse.tile as tile
from concourse.bass_utils import run_bass_kernel_spmd

H = 1024
N_ROWS = 16384
NCORES = 8
P = 128
KB = H // P            # 8 contraction blocks of 128
MB = H // P            # 8 output-feature blocks
NQ = 4                 # k-quarters (256 each)
NS = 512               # rows per matmul moving slice (one PSUM bank)
WSCALE = 32.0          # weight pre-scale (exact power of 2)

# fp8 k-quarters per weight matrix (first nq of 4 quarters are fp8; rest fp16)
NQ8 = {"wz": 2, "uz": 2, "wr": 4, "ur": 4, "wh": 2, "uh": 4}
X16B = KB - 2 * min(NQ8["wz"], NQ8["wh"])   # fp16 x blocks needed (tail blocks)
X16O = KB - X16B                            # first fp16 x block index

F16 = mybir.dt.float16
F8 = mybir.dt.float8e4
F32 = mybir.dt.float32
AF = mybir.ActivationFunctionType
DR = mybir.MatmulPerfMode.DoubleRow
f16 = np.float16
f8 = ml_dtypes.float8_e4m3

# Set by test harness to capture a trace; harness-facing default off.
TRACE = False
LAST_RESULT = None


def build_nc(R=N_ROWS // NCORES):
    """Build the per-core Bass program. R rows per core, single chunk."""
    SL = R // NS           # moving slices (4 for R=2048)

    nc = bacc.Bacc(trn_type="TRN2", target_bir_lowering=False,
                   debug=False, enable_asserts=False)

    # All block tensors use "partition-major block layout": [128, nblk, cols]
    # with element (p, k, c) = T[k*128 + p, c]. One DMA descriptor can then
    # cover any k-block range (descriptor processing on the queue engines
    # costs ~0.65us each — fine-grained DMA was the startup limiter).
    x8d = nc.dram_tensor("x8", [P, KB * R], F8, kind="ExternalInput").ap()
    h8d = nc.dram_tensor("h8", [P, KB * R], F8, kind="ExternalInput").ap()
    x16d = nc.dram_tensor("x16", [P, X16B * R], F16, kind="ExternalInput").ap()
    h16d = nc.dram_tensor("h16", [P, KB * R], F16, kind="ExternalInput").ap()
    w8d = {}
    w16d = {}
    for nm, nq in NQ8.items():
        w8d[nm] = nc.dram_tensor(nm + "8", [P, 2 * nq * H], F8,
                                 kind="ExternalInput").ap()
        if nq < NQ:
            w16d[nm] = nc.dram_tensor(nm + "f", [P, 2 * (NQ - nq) * H], F16,
                                      kind="ExternalInput").ap()
    bias = nc.dram_tensor("bias", [P, 3 * MB], F32, kind="ExternalInput").ap()
    outT = nc.dram_tensor("outT", [H, R], F16, kind="ExternalOutput").ap()

    with tile.TileContext(nc) as tc, ExitStack() as ctx:
        wpool = ctx.enter_context(tc.tile_pool(name="w", bufs=3))
        dpool = ctx.enter_context(tc.tile_pool(name="d", bufs=1))
        rpool = ctx.enter_context(tc.tile_pool(name="r", bufs=3))
        gpool = ctx.enter_context(tc.tile_pool(name="g", bufs=3))
        dtpool = ctx.enter_context(tc.tile_pool(name="dt", bufs=SL))
        opool = ctx.enter_context(tc.tile_pool(name="o", bufs=4))
        cpool = ctx.enter_context(tc.tile_pool(name="c", bufs=1))
        pspool = ctx.enter_context(tc.tile_pool(name="ps", bufs=8, space="PSUM"))

        # Warm up the ACT table set FIRST (sigmoid_and_others covers tanh
        # too), before any scalar-engine DMA doorbells: the DMA ring is only
        # ~5 deep, so doorbells beyond that block the issuing engine until
        # transfers drain — an ACT emitted after them fires ~14us late and
        # stalls the whole r-pass psum recycle. Here it only waits on the
        # gpsimd memset (one sem wait — walrus can't attach the
        # PSEUDO_LOAD_ACT_FUNC_SET to an activation carrying two).
        warm = cpool.tile([P, 8], F32, tag="warm")
        nc.gpsimd.memset(warm[:], 0.0)
        nc.scalar.activation(warm[:], warm[:], AF.Sigmoid)

        bt = cpool.tile([P, 3 * MB], F32, tag="bias")
        nc.sync.dma_start(bt[:], bias[:])
        # bias column layout: [z:0..7 | r:8..15 | h:16..23]
        GZ, GR, GH = 0, 1, 2
        ISC = 1.0 / WSCALE

        # ---- SBUF data tiles ----
        xt8 = dpool.tile([P, KB, R], F8, tag="x8")
        ht8 = dpool.tile([P, KB, R], F8, tag="h8")
        xt16 = dpool.tile([P, X16B, R], F16, tag="x16")
        ht16 = dpool.tile([P, KB, R], F16, tag="h16")
        rht = dpool.tile([P, KB, R], F8, tag="rh")

        w8t = {}
        w16t = {}
        # Critical path (r-pass m=0): the critical 6MB (wr+ur+x8+h8) is
        # split across the two HW DGE queues in consumption order of the
        # first psum group (sync: wr+h8, scalar: x8+ur), with x8's kq0
        # additionally split into s-slices so the first matmul only waits
        # on 128KB. The gpsimd SWDGE queue is NOT used for data: its
        # software descriptor generation is far too slow.
        w8t["wr"] = wpool.tile([P, KB, H], F8, tag="w8", name="wr8", bufs=2)
        w8t["ur"] = wpool.tile([P, KB, H], F8, tag="w8", name="ur8", bufs=2)
        for kq in range(NQ):
            j = slice(2 * kq, 2 * kq + 2)
            nc.sync.dma_start(w8t["wr"][:, j, :],
                              w8d["wr"][:, 2 * kq * H:(2 * kq + 2) * H])
            nc.scalar.dma_start(xt8[:, j, :],
                                x8d[:, 2 * kq * R:(2 * kq + 2) * R])
            nc.sync.dma_start(ht8[:, j, :],
                              h8d[:, 2 * kq * R:(2 * kq + 2) * R])
            nc.scalar.dma_start(w8t["ur"][:, j, :],
                                w8d["ur"][:, 2 * kq * H:(2 * kq + 2) * H])

        # fp16 h stream (rht multiply from m=0 at ~23us, combine later) on
        # the sync ring: the sync ENGINE only issues doorbells, so blocking
        # on ring space there is free. The scalar engine must stay at <=9
        # upfront doorbells or its later ACT instructions get stuck behind
        # ring-full doorbell stalls (that cost 20us in an earlier rev).
        for m in range(MB):
            nc.sync.dma_start(ht16[:, m, :], h16d[:, m * R:(m + 1) * R])

        # hz-pass weights + fp16 x: streamed during the r-pass.
        for nm in ("wh", "wz", "uz"):
            nq = NQ8[nm]
            w8t[nm] = wpool.tile([P, 2 * nq, H], F8, tag="w8q", name=nm + "8")
            nc.sync.dma_start(w8t[nm][:, :, :], w8d[nm][:, :])
            w16t[nm] = wpool.tile([P, 2 * (NQ - nq), H], F16, tag="wfq",
                                  name=nm + "f")
            nc.sync.dma_start(w16t[nm][:, :, :], w16d[nm][:, :])
        nc.scalar.dma_start(xt16[:, :, :], x16d[:, :])
        # uh8 reuses wr's buffer (tag w8, bufs=2): second physical buffer is
        # free immediately; keep it at the sync-queue tail so it can't
        # head-of-line block the critical prefetch above.
        w8t["uh"] = wpool.tile([P, KB, H], F8, tag="w8", name="uh8", bufs=2)
        nc.sync.dma_start(w8t["uh"][:, :, :], w8d["uh"][:, :])

        # ---- PE p-state warmup ----
        # The PE ramps 0.65 -> 1.2 -> 2.4GHz over ~3us of continuous work.
        # Burn the ramp on dummy matmuls over zeroed tiles while the first
        # real DMAs are still in flight. They accumulate into the first real
        # PSUM tile as complete start/stop groups, so the real group's
        # start=True afterwards is clean.
        wuw = cpool.tile([P, P], F16, tag="wuw")
        wum = cpool.tile([P, NS], F16, tag="wum")
        nc.gpsimd.memset(wuw[:], 0.0)
        nc.gpsimd.memset(wum[:], 0.0)

        pss01 = [[pspool.tile([P, NS], F32, tag="ps", name="ps")
                  for _ in range(SL)] for _ in range(2)]
        for _ in range(6):
            nc.tensor.matmul(pss01[0][0][:], wuw[:], wum[:],
                             start=True, stop=True, skip_group_check=True)

        def pe_filler(ps, n):
            """Keep the PE p-state ramp alive while the startup DMA stream
            catches up: accumulate all-zero products into the live psum
            group (numerically a no-op, 216ns each)."""
            for _ in range(n):
                nc.tensor.matmul(ps[:], wuw[:], wum[:], start=False,
                                 stop=False, skip_group_check=True)

        def mm_fp8(psums, wt, mov, m, nq, start, stop):
            """DoubleRow-accumulate wt.T @ mov for feature block m over
            fp8 k-quarters 0..nq-1."""
            msl = slice(m * P, (m + 1) * P)
            for kq in range(nq):
                for s in range(len(psums)):
                    nc.tensor.matmul(
                        psums[s][:],
                        wt[:, 2 * kq:2 * kq + 2, msl],
                        mov[:, 2 * kq:2 * kq + 2, s * NS:(s + 1) * NS],
                        start=start and kq == 0,
                        stop=stop and kq == nq - 1,
                        perf_mode=DR,
                    )

        def mm_f16(psums, wt, mov, m, nk, start, stop, mov_off=0):
            """fp16-accumulate over nk k-blocks of 128. mov_off: first
            k-block of this weight's fp16 span within the mov tile."""
            msl = slice(m * P, (m + 1) * P)
            for k in range(nk):
                for s in range(len(psums)):
                    nc.tensor.matmul(
                        psums[s][:],
                        wt[:, k, msl],
                        mov[:, mov_off + k, s * NS:(s + 1) * NS],
                        start=start and k == 0,
                        stop=stop and k == nk - 1,
                    )

        # ---- r pass ----
        # wr/ur interleaved per kq: matches the arrival order of the DMA
        # queues so the m-groups consume data as it lands. The FIRST two
        # m-blocks are fused into one kq-interleaved wave: during the 0-17us
        # window the critical 6MB is still streaming in and a single m-group
        # (6.9us of matmuls) cannot cover the delivery time; two can. Later
        # groups stay single-m so their ACT drain pipelines under the next
        # group's matmuls (4+4 PSUM bank split — fusing ALL pairs regresses).
        def r_mms(ms, pss, fill=False):
            for kq in range(NQ):
                j = slice(2 * kq, 2 * kq + 2)
                for mi, m in enumerate(ms):
                    msl = slice(m * P, (m + 1) * P)
                    for s in range(SL):
                        nc.tensor.matmul(
                            pss[mi][s][:], w8t["wr"][:, j, msl],
                            xt8[:, j, s * NS:(s + 1) * NS],
                            start=kq == 0, stop=False, perf_mode=DR)
                if fill:
                    pe_filler(pss[0][0], 2)
                for mi, m in enumerate(ms):
                    msl = slice(m * P, (m + 1) * P)
                    for s in range(SL):
                        nc.tensor.matmul(
                            pss[mi][s][:], w8t["ur"][:, j, msl],
                            ht8[:, j, s * NS:(s + 1) * NS],
                            start=False, stop=kq == NQ - 1, perf_mode=DR)
                if fill and kq < NQ - 1:
                    pe_filler(pss[0][0], 2)

        def r_acts(ms, pss):
            for mi, m in enumerate(ms):
                for s in range(SL):
                    rt = rpool.tile([P, NS], F16, tag="r")
                    nc.scalar.activation(rt[:], pss[mi][s][:], AF.Sigmoid,
                                         bias=bt[:, GR * MB + m: GR * MB + m + 1],
                                         scale=ISC)
                    nc.vector.tensor_mul(
                        rht[:, m, s * NS:(s + 1) * NS], rt[:],
                        ht16[:, m, s * NS:(s + 1) * NS])

        r_mms([0, 1], pss01, fill=True)
        r_acts([0, 1], pss01)
        for m in range(2, MB):
            ps = [pspool.tile([P, NS], F32, tag="ps", name="ps") for _ in range(SL)]
            r_mms([m], [ps])
            r_acts([m], [ps])

        # ---- fused h~ / z pass + combine ----
        NF16 = {nm: 2 * (NQ - NQ8[nm]) for nm in NQ8}   # fp16 k-blocks
        for m in range(MB):
            msl = slice(m * P, (m + 1) * P)

            psA = [pspool.tile([P, NS], F32, tag="ps", name="psA") for _ in range(SL)]
            mm_fp8(psA, w8t["wh"], xt8, m, NQ8["wh"], start=True, stop=False)
            mm_f16(psA, w16t["wh"], xt16, m, NF16["wh"], start=False,
                   stop=False, mov_off=2 * NQ8["wh"] - X16O)
            mm_fp8(psA, w8t["uh"], rht, m, NQ8["uh"], start=False, stop=True)
            dts = []
            for s in range(SL):
                ssl = slice(s * NS, (s + 1) * NS)
                gt = gpool.tile([P, NS], F16, tag="g")
                nc.scalar.activation(gt[:], psA[s][:], AF.Tanh,
                                     bias=bt[:, GH * MB + m: GH * MB + m + 1],
                                     scale=ISC)
                # g - h does not depend on z: hoist it ahead of the z matmuls
                dt = dtpool.tile([P, NS], F32, tag="dt")
                nc.vector.tensor_sub(dt[:], gt[:], ht16[:, m, ssl])
                dts.append(dt)

            if m < MB - 1:
                psB = [pspool.tile([P, NS], F32, tag="ps", name="psB")
                       for _ in range(SL)]
                mm_fp8(psB, w8t["wz"], xt8, m, NQ8["wz"], start=True, stop=False)
                mm_f16(psB, w16t["wz"], xt16, m, NF16["wz"], start=False,
                       stop=False, mov_off=2 * NQ8["wz"] - X16O)
                mm_fp8(psB, w8t["uz"], ht8, m, NQ8["uz"], start=False, stop=False)
                mm_f16(psB, w16t["uz"], ht16, m, NF16["uz"], start=False,
                       stop=True, mov_off=2 * NQ8["uz"])
                for s in range(SL):
                    ssl = slice(s * NS, (s + 1) * NS)
                    zt = rpool.tile([P, NS], F16, tag="z")
                    nc.scalar.activation(zt[:], psB[s][:], AF.Sigmoid,
                                         bias=bt[:, GZ * MB + m: GZ * MB + m + 1],
                                         scale=ISC)
                    ot = opool.tile([P, NS], F16, tag="o")
                    # z*(g-h) ; h + z*(g-h)
                    nc.vector.tensor_mul(dts[s][:], zt[:], dts[s][:])
                    nc.vector.tensor_add(ot[:], dts[s][:], ht16[:, m, ssl])
                    nc.sync.dma_start(outT[msl, ssl], ot[:])
            else:
                # last m: complete each s-slice fully so the z->combine->dma
                # chains drain during (not after) the matmul stream; the last
                # s-slice is split in half so the final drain chain is short.
                def z_chain(ps, csl, width):
                    for kq in range(NQ8["wz"]):
                        nc.tensor.matmul(ps[:], w8t["wz"][:, 2*kq:2*kq+2, msl],
                                         xt8[:, 2*kq:2*kq+2, csl],
                                         start=kq == 0, stop=False,
                                         perf_mode=DR)
                    for k in range(NF16["wz"]):
                        nc.tensor.matmul(ps[:], w16t["wz"][:, k, msl],
                                         xt16[:, 2*NQ8["wz"]-X16O+k, csl],
                                         start=False, stop=False)
                    for kq in range(NQ8["uz"]):
                        nc.tensor.matmul(ps[:], w8t["uz"][:, 2*kq:2*kq+2, msl],
                                         ht8[:, 2*kq:2*kq+2, csl],
                                         start=False, stop=False, perf_mode=DR)
                    for k in range(NF16["uz"]):
                        nc.tensor.matmul(ps[:], w16t["uz"][:, k, msl],
                                         ht16[:, 2*NQ8["uz"]+k, csl],
                                         start=False, stop=k == NF16["uz"]-1)
                    zt = rpool.tile([P, width], F16, tag="z")
                    nc.scalar.activation(zt[:], ps[:], AF.Sigmoid,
                                         bias=bt[:, GZ * MB + m: GZ * MB + m + 1],
                                         scale=ISC)
                    s = csl.start // NS
                    dsl = slice(csl.start - s * NS, csl.start - s * NS + width)
                    ot = opool.tile([P, width], F16, tag="o")
                    nc.vector.tensor_mul(dts[s][:, dsl], zt[:], dts[s][:, dsl])
                    nc.vector.tensor_add(ot[:], dts[s][:, dsl], ht16[:, m, csl])
                    nc.sync.dma_start(outT[msl, csl], ot[:])

                for s in range(SL - 1):
                    ps = pspool.tile([P, NS], F32, tag="ps", name="psB")
                    z_chain(ps[:], slice(s * NS, (s + 1) * NS), NS)
                # final slice: two sequential half-bank groups so the very
                # last act->combine->dma chain is half as long
                HNS = NS // 2
                ps = pspool.tile([P, NS], F32, tag="ps", name="psB")
                for hf in range(2):
                    c0 = (SL - 1) * NS + hf * HNS
                    z_chain(ps[:, hf * HNS:(hf + 1) * HNS],
                            slice(c0, c0 + HNS), HNS)

    nc.compile()
    return nc


_NC_CACHE = {}


def _get_nc(R):
    if R not in _NC_CACHE:
        _NC_CACHE[R] = build_nc(R)
    return _NC_CACHE[R]


def blockify(a):
    """[nb*128, C] -> partition-major block layout [128, nb*C]."""
    nb = a.shape[0] // P
    return np.ascontiguousarray(
        a.reshape(nb, P, -1).transpose(1, 0, 2).reshape(P, -1))


def make_in_maps(update, hidden, wz, uz, bz, wr, ur, br, wh, uh, bh,
                 ncores=NCORES):
    wmap = {}
    for nm, w in (("wz", wz), ("uz", uz), ("wr", wr), ("ur", ur),
                  ("wh", wh), ("uh", uh)):
        wT = np.ascontiguousarray(np.asarray(w, np.float32).T) * WSCALE
        nq = NQ8[nm]
        wmap[nm + "8"] = blockify(wT[:nq * 2 * P].astype(f8))
        if nq < NQ:
            wmap[nm + "f"] = blockify(wT[nq * 2 * P:].astype(f16))
    bias = np.empty((P, 3 * MB), np.float32)
    for g, b in enumerate((bz, br, bh)):
        bias[:, g * MB:(g + 1) * MB] = np.asarray(b, np.float32).reshape(MB, P).T
    rows = update.shape[0]
    rc = rows // ncores
    in_maps = []
    for i in range(ncores):
        sl = slice(i * rc, (i + 1) * rc)
        xT = np.ascontiguousarray(np.asarray(update[sl], np.float32).T)
        hT = np.ascontiguousarray(np.asarray(hidden[sl], np.float32).T)
        in_maps.append(dict(
            x8=blockify(xT.astype(f8)), x16=blockify(xT[X16O * P:].astype(f16)),
            h8=blockify(hT.astype(f8)), h16=blockify(hT.astype(f16)),
            bias=bias, **wmap))
    return in_maps


def kernel(update, hidden, wz, uz, bz, wr, ur, br, wh, uh, bh):
    global LAST_RESULT
    update = np.asarray(update)
    hidden = np.asarray(hidden)
    R = update.shape[0] // NCORES
    nc = _get_nc(R)
    in_maps = make_in_maps(update, hidden, wz, uz, bz, wr, ur, br, wh, uh, bh)
    res = run_bass_kernel_spmd(nc, in_maps, list(range(NCORES)), trace=TRACE)
    LAST_RESULT = res
    out = np.empty((update.shape[0], H), np.float32)
    for i in range(NCORES):
        out[i * R:(i + 1) * R] = res.results[i]["outT"].T
    return out
